# revision 2
# baseline (speedup 1.0000x reference)
"""DistMult edge scoring on Trainium2 (8 NeuronCores).

score_e = src_emb[e]^T @ W[rel_e] @ dst_emb[e]   for E=100k edges.

Strategy (v2 — compact-table + on-chip ap_gather)
-------------------------------------------------
Host (index-space preprocessing; embedding rows are only *compacted*,
never expanded per-edge):
  - Sort edges by relation, shard the sorted list contiguously across
    the 8 cores (data-parallel over edges). Pad each relation run to a
    multiple of 128 so every 128-edge tile is single-relation.
  - Per core, the set of node ids referenced by its edges is <= 25k
    (< 2^15), so a per-core COMPACT table indexed by int16 replaces the
    100k-row global table. The table is stored transposed [64 dims, U]
    in first-use order (so early slots only need an early-column
    prefix), and mirrored to partitions 64..127.
  - src/dst ids are remapped to compact-table columns; per 1024-slot
    chunk the max referenced column is recorded so the device gather
    only waits on that prefix of the table DMA.

Device (per core, SPMD):
  - Contiguous HWDGE DMA only: idx tile, compact table (column chunks),
    per-tile W, scores out. No SWDGE descriptor-per-row gathers at all.
  - Per 1024-slot chunk, ONE gpsimd.ap_gather (channels=128) gathers
    srcT [64 dims, 1024 slots] into partitions 0..63 (from the lower
    table copy) and dstT into partitions 64..127 (mirror) — all 8 Q7
    cores active, output already transposed for the matmul.
  - Per tile j: V[k,e] = sum_d W[r_j][d,k] * srcT[d,e]  (PE matmul,
    lhsT = W tile, rhs = srcT tile).
  - Per chunk: VD = V * dstT (DVE), then scores[1, e] = sum_k VD[k, e]
    via a ones-vector matmul (PE), ACT-copied into a flat scores row.
Host: drop pad slots, unsort scores to the original edge order.
"""

import numpy as np

import concourse.bacc as bacc
import concourse.mybir as mybir
from concourse.bass_utils import run_bass_kernel_spmd

NCORES = 8
P = 128          # edges per tile
DIM = 64         # embedding dim
TILE_GROUP = 8   # tiles per compute chunk (1024 slots)
CHUNK = TILE_GROUP * P

TRACE = False
LAST_RESULT = None

_BUILD_CACHE = {}


def _prepare(triplets, num_nodes):
    """Index-space prep. Returns per-core idx tiles, compact-table maps,
    per-chunk table-prefix bounds, and the unsort map."""
    t = np.asarray(triplets)
    E = t.shape[0]
    src = t[:, 0].astype(np.int64)
    rel = t[:, 1].astype(np.int64)
    dst = t[:, 2].astype(np.int64)

    order = np.argsort(rel, kind="stable")
    bounds = [round(c * E / NCORES) for c in range(NCORES + 1)]

    # --- per-core slot layout: relation runs padded to P ---
    core_se = []
    core_rel = []
    K = 0
    for c in range(NCORES):
        eidx = order[bounds[c]:bounds[c + 1]]
        r = rel[eidx]
        n = len(eidx)
        change = np.flatnonzero(np.diff(r)) + 1
        starts = np.concatenate([[0], change])
        ends = np.concatenate([change, [n]])
        lens = ends - starts
        padlens = ((lens + P - 1) // P) * P
        offs = np.concatenate([[0], np.cumsum(padlens)])
        total = int(offs[-1])
        se = np.full(total, -1, np.int64)
        pos = (np.arange(n) - np.repeat(starts, lens)
               + np.repeat(offs[:-1], lens))
        se[pos] = eidx
        sr = np.repeat(r[starts], padlens)
        core_se.append(se)
        core_rel.append(sr)
        K = max(K, total // P)
    K = -(-K // TILE_GROUP) * TILE_GROUP
    nslot = K * P

    slot_edge = np.full((NCORES, nslot), -1, np.int64)
    relc_all = np.zeros((NCORES, K), np.int64)
    sloc_all = np.zeros((NCORES, nslot), np.int64)
    dloc_all = np.zeros((NCORES, nslot), np.int64)
    uniq_all = []
    U = 0
    nchunk = K // TILE_GROUP
    xmax = np.zeros((NCORES, nchunk), np.int64)

    for c in range(NCORES):
        se, sr = core_se[c], core_rel[c]
        m = len(se)
        slot_edge[c, :m] = se
        nt = m // P
        relc_all[c, :nt] = sr.reshape(nt, P)[:, 0]
        valid = se >= 0
        # first-use order over the (src0, dst0, src1, dst1, ...) walk
        ids = np.empty(2 * m, np.int64)
        ids[0::2] = np.where(valid, src[se], -1)
        ids[1::2] = np.where(valid, dst[se], -1)
        seq = ids[ids >= 0]
        _, first = np.unique(seq, return_index=True)
        uniq = seq[np.sort(first)]
        Uc = len(uniq)
        assert Uc <= 32767
        lut = np.zeros(num_nodes, np.int64)
        lut[uniq] = np.arange(Uc)
        sl = np.zeros(nslot, np.int64)
        dl = np.zeros(nslot, np.int64)
        sl[:m][valid] = lut[src[se[valid]]]
        dl[:m][valid] = lut[dst[se[valid]]]
        sloc_all[c] = sl
        dloc_all[c] = dl
        uniq_all.append(uniq)
        U = max(U, Uc)
        hi = np.maximum(sl, dl).reshape(nchunk, CHUNK).max(axis=1)
        xmax[c] = hi
    U = -(-U // 16) * 16

    # per-chunk table prefix (same for all cores — SPMD program)
    X = np.maximum.accumulate(xmax.max(axis=0)) + 1
    X = np.minimum(-(-X // 16) * 16, U)
    X = tuple(int(x) for x in X)

    # table DMA column chunks: fine-grained at the head, coarse later
    cuts = [0]
    for step in (2048, 2048, 4096, 4096, 8192, 8192):
        if cuts[-1] + step >= U:
            break
        cuts.append(cuts[-1] + step)
    cuts.append(U)
    tab_cuts = tuple(cuts)

    def tile16(flat):
        # wrap into 16 partitions: idx for slot s at [s % 16, s // 16]
        return flat.reshape(-1, 16).T.astype(np.int16)

    fidx = np.zeros((NCORES, 128, nslot // 16), np.int16)
    for c in range(NCORES):
        s16 = tile16(sloc_all[c])
        d16 = tile16(dloc_all[c])
        fidx[c, 0:64] = np.tile(s16, (4, 1))
        fidx[c, 64:128] = np.tile(d16, (4, 1))

    return fidx, relc_all, uniq_all, slot_edge, K, U, X, tab_cuts, E


def _build(K, U, X, tab_cuts):
    nc = bacc.Bacc("TRN2", target_bir_lowering=False, debug=False,
                   num_devices=NCORES)
    from concourse.tile import TileContext
    f32, i16 = mybir.dt.float32, mybir.dt.int16
    nslot = K * P
    nchunk = K // TILE_GROUP

    table_d = nc.dram_tensor("tableT", [128, U], f32, kind="ExternalInput")
    wt_d = nc.dram_tensor("w_tile", [DIM, K * DIM], f32,
                          kind="ExternalInput")
    fidx_d = nc.dram_tensor("fidx", [128, nslot // 16], i16,
                            kind="ExternalInput")
    out_d = nc.dram_tensor("scores", [1, nslot], f32, kind="ExternalOutput")

    with TileContext(nc) as tc:
        with (
            tc.tile_pool(name="persist", bufs=1) as persist,
            tc.tile_pool(name="g", bufs=3) as g_pool,
            tc.tile_pool(name="vd", bufs=2) as vd_pool,
            tc.tile_pool(name="vpsum", bufs=2, space="PSUM") as vpsum_pool,
            tc.tile_pool(name="spsum", bufs=2, space="PSUM") as spsum_pool,
        ):
            fidx = persist.tile([128, nslot // 16], i16, tag="fidx")
            table = persist.tile([128, U], f32, tag="table")
            w_g = persist.tile([DIM, K * DIM], f32, tag="w_g")
            ones = persist.tile([DIM, 1], f32, tag="ones")
            scores = persist.tile([1, nslot], f32, tag="scores")

            nc.sync.dma_start(out=fidx[:], in_=fidx_d[:])
            nc.vector.memset(ones[:], 1.0)

            # table column chunks (prefix-ordered), W interleaved so the
            # first supertiles' weights arrive early
            nwc = [0, 2 * TILE_GROUP * DIM, 6 * TILE_GROUP * DIM, K * DIM]
            nc.sync.dma_start(out=table[:, tab_cuts[0]:tab_cuts[1]],
                              in_=table_d[:, tab_cuts[0]:tab_cuts[1]])
            nc.sync.dma_start(out=w_g[:, nwc[0]:nwc[1]],
                              in_=wt_d[:, nwc[0]:nwc[1]])
            nc.sync.dma_start(out=table[:, tab_cuts[1]:tab_cuts[2]],
                              in_=table_d[:, tab_cuts[1]:tab_cuts[2]])
            nc.sync.dma_start(out=w_g[:, nwc[1]:nwc[2]],
                              in_=wt_d[:, nwc[1]:nwc[2]])
            for i in range(2, len(tab_cuts) - 1):
                nc.sync.dma_start(out=table[:, tab_cuts[i]:tab_cuts[i + 1]],
                                  in_=table_d[:, tab_cuts[i]:tab_cuts[i + 1]])
            nc.sync.dma_start(out=w_g[:, nwc[2]:nwc[3]],
                              in_=wt_d[:, nwc[2]:nwc[3]])

            for st in range(nchunk):
                x = X[st]
                gt = g_pool.tile([128, CHUNK], f32, tag="g")
                nc.gpsimd.ap_gather(
                    out_ap=gt[:].rearrange("p (n d) -> p n d", d=1),
                    in_ap=table[:, :x].rearrange("p (n d) -> p n d", d=1),
                    idxs_ap=fidx[:, st * (CHUNK // 16):(st + 1) * (CHUNK // 16)],
                    channels=128,
                    num_elems=x,
                    d=1,
                    num_idxs=CHUNK,
                )
                vps = vpsum_pool.tile([DIM, CHUNK], f32, tag="vps")
                for h in range(TILE_GROUP):
                    j = st * TILE_GROUP + h
                    nc.tensor.matmul(
                        out=vps[:, h * P:(h + 1) * P],
                        lhsT=w_g[:, j * DIM:(j + 1) * DIM],
                        rhs=gt[0:DIM, h * P:(h + 1) * P],
                        start=True,
                        stop=True,
                    )
                vdt = vd_pool.tile([DIM, CHUNK], f32, tag="vd")
                nc.vector.tensor_mul(
                    out=vdt[:],
                    in0=vps[:],
                    in1=gt[DIM:128, :],
                )
                for half in range(2):
                    sps = spsum_pool.tile([1, CHUNK // 2], f32, tag="sps")
                    nc.tensor.matmul(
                        out=sps[:],
                        lhsT=ones[:],
                        rhs=vdt[:, half * (CHUNK // 2):(half + 1) * (CHUNK // 2)],
                        start=True,
                        stop=True,
                    )
                    o0 = st * CHUNK + half * (CHUNK // 2)
                    nc.scalar.copy(out=scores[:, o0:o0 + CHUNK // 2],
                                   in_=sps[:])

            nc.sync.dma_start(out=out_d[:], in_=scores[:])

    nc.compile()
    return nc


def kernel(triplets, node_emb, W):
    global LAST_RESULT
    node = np.ascontiguousarray(np.asarray(node_emb, dtype=np.float32))
    Wf = np.ascontiguousarray(np.asarray(W, dtype=np.float32))
    num_nodes = node.shape[0]

    (fidx, relc_all, uniq_all, slot_edge, K, U, X, tab_cuts, E) = \
        _prepare(triplets, num_nodes)

    cache_key = (K, U, X, tab_cuts)
    if cache_key not in _BUILD_CACHE:
        _BUILD_CACHE[cache_key] = _build(K, U, X, tab_cuts)
    nc = _BUILD_CACHE[cache_key]

    in_maps = []
    for c in range(NCORES):
        uniq = uniq_all[c]
        tabT = np.zeros((128, U), np.float32)
        tt = node[uniq].T  # [64, Uc]
        tabT[0:DIM, :tt.shape[1]] = tt
        tabT[DIM:128, :tt.shape[1]] = tt
        wt = np.ascontiguousarray(
            Wf[relc_all[c]].transpose(1, 0, 2).reshape(DIM, K * DIM))
        in_maps.append({
            "tableT": tabT,
            "w_tile": wt,
            "fidx": np.ascontiguousarray(fidx[c]),
        })

    res = run_bass_kernel_spmd(nc, in_maps, list(range(NCORES)), trace=TRACE)
    LAST_RESULT = res

    out = np.zeros(E, np.float32)
    for c in range(NCORES):
        flat = np.asarray(res.results[c]["scores"]).ravel()  # [nslot]
        se = slot_edge[c]
        valid = se >= 0
        out[se[valid]] = flat[valid]
    return out
